# revision 39
# baseline (speedup 1.0000x reference)
"""GQA attention (RoPE, causal, per-head q-scale) on 8 TRN2 NeuronCores.

Sharding: 2-way data-parallel over batch x 4-way tensor-parallel over heads.
Core c handles batch b=c//4 and head group g=c%4 (8 q heads, 2 kv heads).
Each core computes qkv-proj -> rope -> causal attention -> partial o_proj
(over its heads' columns of Wo); the host sums the 4 partials per batch.

All scalar factors (rope_mscale, sm_scale, per_head_scale) are folded into
the Wq/Wk rows on the host. Causal masking: fully-masked column blocks are
skipped (matmul widths trimmed to the causal extent); diagonal blocks get
-BIG added in PSUM via one identity x [tri|tri] matmul covering both heads
before the exp.

dtypes: the whole device datapath is fp16 (better mantissa than bf16 at the
same matmul/DVE speed); PSUM accumulation is f32.  exp uses a -5 bias so
probabilities stay inside fp16 range (cancels in the softmax ratio).
RoPE and normalization run on fp16 SBUF operands so the DVE gets its 2x
packed mode; psum->sbuf stagings ride the scalar engine.

Emission is software-pipelined: attention(j) head-pairs are interleaved
with QKV(j+1) chunks and o_proj(j-1) row blocks so the in-order PE queue
always has independent work while the scalar engine streams the exps.

Layouts on device (partition, free):
  xt      [hid, s]        hidden^T, streamed in 512-col chunks
  wqkv    [hid, 768]      [Wq(8 heads, scaled) | Wk(2 kv, scaled) | Wv].T
  q/k^T   [d*heads, s]    head-major rows; rope applied in this layout
  scores^T[sk, 2, sq]     per (pair, sk-chunk 128, sq-chunk 512) in PSUM
  exp^T   [sk, 2, sq]     SBUF fp16, fed as matmul rhs (both heads packed)
  Vaug    [sk, 128]       V rows (0:64) + 64 ones cols; PV matmul output
                          rows 64:128 then hold the softmax denominators
                          already broadcast over 64 partitions
  out^T   [2d, 2, sq]     PSUM accumulator per (pair, sq-chunk), both heads
  attn^T  [o(=2 heads), s] normalized fp16, lhsT for o_proj
  out     [s, hid_out]    partial o_proj result (fp16), one per core
"""

import sys, os

for _p in ("/opt/trn_rl_repo", "/root/.axon_site/_ro/trn_rl_repo"):
    if os.path.isdir(_p) and _p not in sys.path:
        sys.path.insert(0, _p)

import numpy as np

import concourse.bass as bass
import concourse.mybir as mybir
import concourse.tile as tile
from concourse import bacc
from concourse.bass_utils import run_bass_kernel_spmd

F32 = mybir.dt.float32
F16 = mybir.dt.float16
AF = mybir.ActivationFunctionType

B, S, HID = 2, 2048, 2048
H, K, D = 32, 8, 64
G = H // K
ROPE_MSCALE = 1.2
SM_SCALE = 1.0 / (D ** 0.5)
BIG = 30000.0
EXP_BIAS = -5.0  # exp(s-5) keeps fp16 probs finite; cancels in softmax

NH = 8           # q heads per core
NKV = 2          # kv heads per core
NPAIR = 4        # q head pairs per core
QO = NH * D      # 512 q rows
NK = HID // 128  # 16 contraction chunks
SQW = 512        # sq / xt chunk width
NJ = S // SQW    # 4 chunks
NSK = S // 128   # 16 sk chunks

_CACHED = {}


def _build():
    if "nc" in _CACHED:
        return _CACHED["nc"]

    nc = bacc.Bacc(None)

    xt_d = nc.declare_dram_parameter("xt", [HID, S], F16, isOutput=False)
    wqkv_d = nc.declare_dram_parameter("wqkv", [HID, 768], F16, isOutput=False)
    wo_d = nc.declare_dram_parameter("wo", [QO, HID], F16, isOutput=False)
    cost_d = nc.declare_dram_parameter("cost", [128, S], F16, isOutput=False)
    sints_d = nc.declare_dram_parameter("sints", [128, S], F16, isOutput=False)
    consts_d = nc.declare_dram_parameter("consts", [128, 386], F16, isOutput=False)
    out_d = nc.declare_dram_parameter("out", [S, HID], F16, isOutput=True)

    with tile.TileContext(nc) as tc:
        # ---------- long-lived pools ----------
        with (
            tc.tile_pool(name="consts", bufs=1) as consts_pool,
            tc.tile_pool(name="ktv", bufs=1) as ktv_pool,
            tc.tile_pool(name="qrope", bufs=10) as qrope_pool,
            tc.tile_pool(name="expt", bufs=6) as expt_pool,
            tc.tile_pool(name="attnt", bufs=8) as attnt_pool,
            tc.tile_pool(name="inv", bufs=2) as inv_pool,
            tc.tile_pool(name="wo", bufs=1) as wo_pool,
            tc.tile_pool(name="ost", bufs=6) as ost_pool,
            tc.tile_pool(name="wq", bufs=1) as wq_pool,
            tc.tile_pool(name="xt", bufs=8) as xt_pool,
            tc.tile_pool(name="cs", bufs=1) as cs_pool,
            tc.tile_pool(name="rtmp", bufs=2) as rtmp_pool,
            tc.tile_pool(name="psc", bufs=2, space="PSUM") as psc_pool,
            tc.tile_pool(name="pout2", bufs=1, space="PSUM") as pout2_pool,
            tc.tile_pool(name="pqkv", bufs=2, space="PSUM") as pqkv_pool,
        ):
            xt_r = xt_d.rearrange("(kc p) s -> p kc s", p=128)
            wq_r = wqkv_d.rearrange("(kc p) o -> p kc o", p=128)
            wo_r = wo_d.rearrange("(m p) h -> p m h", p=128)
            xt_tiles = {}

            def load_xt(j, interleave_with=None):
                t = []
                for kq in range(4):
                    tt = xt_pool.tile([128, 4, SQW], F16, tag="xt", name="xtt")
                    nc.sync.dma_start(
                        out=tt,
                        in_=xt_r[:, kq * 4:(kq + 1) * 4, j * SQW:(j + 1) * SQW],
                    )
                    t.append(tt)
                    if interleave_with is not None:
                        interleave_with(kq)
                xt_tiles[j] = t

            # prioritized loads: consts, then wqkv/xt(0) interleaved per 4-k
            # batch so the j=0 projection can chase the arriving DMAs; rope
            # tables mid-stream (needed ~20us in), Wo last.
            consts = consts_pool.tile([128, 386], F16, name="consts")
            nc.sync.dma_start(out=consts, in_=consts_d[:, :])

            # HAM warm-up: matmuls on the (tiny, first-to-arrive) consts tile
            # keep the PE busy during the big loads so the clock gate is at
            # 8/8 when real work arrives (~3.4us of activity needed).
            pwarm = pout2_pool.tile([128, 2, SQW], F32, tag="p2", name="pwarm")
            for _ in range(18):
                nc.tensor.matmul(
                    pwarm[:, 0, 0:384], consts[:, 0:128], consts[:, 0:384],
                    start=True, stop=True, skip_group_check=True,
                )
            costh = cs_pool.tile([128, S], F16, tag="cost", name="costh")
            sinth = cs_pool.tile([128, S], F16, tag="sints", name="sinth")
            wqt = wq_pool.tile([128, NK, 768], F16, name="wqt")
            xt0 = []
            for kq in range(4):
                nc.sync.dma_start(
                    out=wqt[:, kq * 4:(kq + 1) * 4, :],
                    in_=wq_r[:, kq * 4:(kq + 1) * 4, :],
                )
                tt = xt_pool.tile([128, 4, SQW], F16, tag="xt", name="xtt")
                nc.sync.dma_start(out=tt, in_=xt_r[:, kq * 4:(kq + 1) * 4, 0:SQW])
                xt0.append(tt)
                if kq == 1:
                    nc.sync.dma_start(out=costh, in_=cost_d[:, :])
                    nc.sync.dma_start(out=sinth, in_=sints_d[:, :])
            xt_tiles[0] = xt0
            wot = wo_pool.tile([128, NPAIR, HID], F16, name="wot")
            nc.sync.dma_start(out=wot, in_=wo_r)

            identh = consts[:, 0:128]
            trih2 = consts[:, 128:384]  # two side-by-side -BIG lower triangles
            ebias = consts[:, 384:385]  # EXP_BIAS column for the activations

            kt_aa = ktv_pool.tile([128, S], F16, tag="ktaa", name="ktaa")
            kt_bb = ktv_pool.tile([128, S], F16, tag="ktbb", name="ktbb")
            # Vaug: cols 0:64 = V, cols 64:128 = 1.0 (sums -> rows 64:128 of PV out)
            vaug = [
                ktv_pool.tile([128, NSK, 128], F16, tag=f"vaug{i}", name=f"vaug{i}")
                for i in range(NKV)
            ]
            ones_col = consts[:, 385:386]
            for i in range(NKV):
                nc.vector.tensor_copy(
                    vaug[i][:, :, 64:128],
                    ones_col[:, None, :].broadcast_to([128, NSK, 64]),
                )

            qrope = {}   # (m, j) -> tile [128, SQW] f16
            attnt = {}   # (m, j) -> tile [128, SQW] f16
            _pending_pe = []  # deferred PE work (V transposes) to emit
                              # behind other matmuls so their input-staging
                              # latency never stalls the in-order PE queue

            def rope(psum_q, j, dst):
                """RoPE a [128, SQW] projected chunk (2 heads) into dst.
                q' = q*cos + swap_halves(q)*sin_signed.  The chunk is staged
                to fp16 SBUF by the scalar engine so every DVE op runs in the
                2x packed 16-bit mode and the QKV PSUM bank is released
                early.  sinth is partition-permuted + sign-folded on the host
                so both DVE inputs share a base partition (SBUF-SBUF rule);
                only the output is partition-shifted, which the DVE allows."""
                c0, c1 = j * SQW, (j + 1) * SQW
                qs = rtmp_pool.tile([128, SQW], F16, tag="qs", name="qs")
                nc.scalar.copy(qs, psum_q)
                t2 = rtmp_pool.tile([128, SQW], F16, tag="t2", name="t2")
                for base in (0, 64):
                    nc.vector.tensor_mul(
                        t2[base:base + 32, :], qs[base + 32:base + 64, :],
                        sinth[base + 32:base + 64, c0:c1],
                    )
                    nc.vector.tensor_mul(
                        t2[base + 32:base + 64, :], qs[base:base + 32, :],
                        sinth[base:base + 32, c0:c1],
                    )
                t4 = rtmp_pool.tile([128, SQW], F16, tag="t4", name="t4")
                nc.vector.tensor_mul(t4, qs, costh[:, c0:c1])
                nc.vector.tensor_add(dst, t2, t4)

            _done_pairs = set()

            def attention_pair(j, m):
                if (j, m) in _done_pairs:
                    return
                _done_pairs.add((j, m))
                nsk = 4 * (j + 1)
                kt = kt_aa if m < 2 else kt_bb
                va = vaug[m // 2]
                qr = qrope.pop((m, j))
                p2 = pout2_pool.tile([128, 2, SQW], F32, tag="p2", name="p2")
                pend = []  # staged (exp tile, sk)
                for sk in range(nsk):
                    # both heads' scores in one 2-bank tile; band blocks only
                    # compute the causally-valid columns
                    p1 = psc_pool.tile([128, 2, SQW], F32, tag="sc", name="sc")
                    band = sk >= 4 * j
                    off = (sk - 4 * j) * 128 if band else 0
                    for hi, hb in enumerate((0, 64)):
                        nc.tensor.matmul(
                            p1[:, hi, off:SQW],
                            kt[hb:hb + 64, sk * 128:(sk + 1) * 128],
                            qr[hb:hb + 64, off:SQW],
                            start=True,
                            stop=True,
                            skip_group_check=True,
                        )
                    if band:
                        # -BIG triangle onto both heads' diagonal blocks,
                        # added in PSUM by the vector engine (keeps the PE
                        # free; matmul accumulation is done by this point)
                        for hi in (0, 1):
                            nc.vector.tensor_add(
                                p1[:, hi, off:off + 128],
                                p1[:, hi, off:off + 128],
                                trih2[:, 0:128],
                            )
                    et = expt_pool.tile([128, 2, SQW], F16, tag="et", name="et")
                    nc.scalar.activation(
                        et[:, :, off:SQW], p1[:, :, off:SQW],
                        AF.Exp, bias=ebias,
                    )
                    pend.append((et, sk))
                    if sk == 1 and _pending_pe:
                        _pending_pe.pop(0)()
                    while len(pend) > 3 or (sk == nsk - 1 and pend):
                        et2, psk = pend.pop(0)
                        poff = (psk - 4 * j) * 128 if psk >= 4 * j else 0
                        for hi in (0, 1):
                            nc.tensor.matmul(
                                p2[:, hi, poff:SQW],
                                va[:, psk, :],
                                et2[:, hi, poff:SQW],
                                start=(psk == 0),
                                stop=(psk == nsk - 1),
                                skip_group_check=True,
                            )
                # normalize: attnT = out^T * (1/sums); sums come out of the
                # PV matmul pre-broadcast in psum rows 64:128
                # per-head chains: head B's sums copy rides the scalar engine
                # so both heads' recip+mul pipelines overlap on vector
                at = attnt_pool.tile([128, SQW], F16, tag="at", name="at")
                sums = inv_pool.tile([64, 2, SQW], F32, tag="sums", name="sums")
                invb = inv_pool.tile([64, 2, SQW], F32, tag="invb", name="invb")
                nc.vector.tensor_copy(sums[:, 0, :], p2[64:128, 0, :])
                nc.scalar.copy(sums[:, 1, :], p2[64:128, 1, :])
                nc.vector.reciprocal_approx_fast(out=invb[:, 0, :], in_=sums[:, 0, :])
                nc.vector.tensor_mul(at[0:64, :], p2[0:64, 0, :], invb[:, 0, :])
                nc.vector.reciprocal_approx_fast(out=invb[:, 1, :], in_=sums[:, 1, :])
                nc.vector.tensor_mul(at[64:128, :], p2[0:64, 1, :], invb[:, 1, :])
                attnt[(m, j)] = at

            def oproj_sc(j, sc):
                orow = None
                for hc in range(HID // 512):
                    po = pqkv_pool.tile([128, 512], F32, tag="qkv", name="po")
                    for m in range(NPAIR):
                        nc.tensor.matmul(
                            po,
                            attnt[(m, j)][:, sc * 128:(sc + 1) * 128],
                            wot[:, m, hc * 512:(hc + 1) * 512],
                            start=(m == 0),
                            stop=(m == NPAIR - 1),
                        )
                    if orow is None:
                        # allocated just before the first evacuation so the
                        # pool-slot wait binds to the vector copy, not the
                        # o_proj matmuls
                        orow = ost_pool.tile([128, HID], F16, tag="orow", name="orow")
                    if hc % 2 == 0:
                        nc.vector.tensor_copy(orow[:, hc * 512:(hc + 1) * 512], po)
                    else:
                        nc.scalar.copy(orow[:, hc * 512:(hc + 1) * 512], po)
                r0 = j * SQW + sc * 128
                # alternate trigger queues so the tail's bunched row DMAs
                # don't serialize on one engine's DMA queue
                eng = nc.gpsimd if sc % 2 == 0 else nc.sync
                eng.dma_start(out=out_d[r0:r0 + 128, :], in_=orow)
                if sc == SQW // 128 - 1:
                    for m in range(NPAIR):
                        attnt.pop((m, j))

            def qkv_chunk(j, m, xt_t):
                if m == 5:
                    # V projected in the [d, s] orientation (512-col moving,
                    # weight loads hidden), then PE-transposed per 128-block
                    # into the [sk, d] layout Vaug needs.
                    pv = pqkv_pool.tile([128, SQW], F32, tag="qkv", name="pv")
                    for k in range(NK):
                        nc.tensor.matmul(
                            pv,
                            wqt[:, k, 640:768],
                            xt_t[k // 4][:, k % 4, :],
                            start=(k == 0),
                            stop=(k == NK - 1),
                        )
                    vt = rtmp_pool.tile([128, SQW], F16, tag="vt", name="vt")
                    nc.scalar.copy(vt, pv)

                    def _finish_v(j=j, vt=vt):
                        tp = psc_pool.tile([128, 4, 128], F16, tag="sc", name="tp")
                        for blk in range(4):
                            nc.tensor.transpose(
                                tp[:, blk, :], vt[:, blk * 128:(blk + 1) * 128],
                                identh,
                            )
                        sk0 = (j * SQW) // 128
                        for i in range(NKV):
                            for blk in range(4):
                                nc.vector.tensor_copy(
                                    vaug[i][:, sk0 + blk, 0:64],
                                    tp[:, blk, i * 64:(i + 1) * 64],
                                )
                    _pending_pe.append(_finish_v)
                    return
                pq = pqkv_pool.tile([128, SQW], F32, tag="qkv", name="pqkv")
                for k in range(NK):
                    nc.tensor.matmul(
                        pq,
                        wqt[:, k, m * 128:(m + 1) * 128],
                        xt_t[k // 4][:, k % 4, :],
                        start=(k == 0),
                        stop=(k == NK - 1),
                    )
                if m < NPAIR:
                    qrope[(m, j)] = qrope_pool.tile(
                        [128, SQW], F16, tag="qr", name="qr"
                    )
                    rope(pq, j, qrope[(m, j)])
                else:  # m == 4: K
                    kro = rtmp_pool.tile([128, SQW], F16, tag="kro", name="kro")
                    rope(pq, j, kro)
                    c0, c1 = j * SQW, (j + 1) * SQW
                    for dst_b in (0, 64):
                        nc.vector.tensor_copy(
                            kt_aa[dst_b:dst_b + 64, c0:c1], kro[0:64, :]
                        )
                        nc.vector.tensor_copy(
                            kt_bb[dst_b:dst_b + 64, c0:c1], kro[64:128, :]
                        )

            def qkv_j0():
                """j=0 projection with k OUTER across all five chunks so the
                matmuls chase the arriving wqt/xt0 DMA batches.  The five
                f32 accumulators land in five distinct psum banks (the q
                pairs borrow the two psc slots' bank halves), so the
                interleaved starts never clear each other's has_written."""
                pq4 = pqkv_pool.tile([128, SQW], F32, tag="qkv", name="pqkv")
                pA = psc_pool.tile([128, 2, SQW], F32, tag="sc", name="scA")
                pB = psc_pool.tile([128, 2, SQW], F32, tag="sc", name="scB")
                accs = [pA[:, 0, :], pA[:, 1, :], pB[:, 0, :], pB[:, 1, :]]
                xt_t = xt_tiles[0]
                for k in range(NK):
                    nc.tensor.matmul(
                        pq4,
                        wqt[:, k, 512:640],
                        xt_t[k // 4][:, k % 4, :],
                        start=(k == 0),
                        stop=(k == NK - 1),
                        skip_group_check=True,
                    )
                    for m in range(NPAIR):
                        nc.tensor.matmul(
                            accs[m],
                            wqt[:, k, m * 128:(m + 1) * 128],
                            xt_t[k // 4][:, k % 4, :],
                            start=(k == 0),
                            stop=(k == NK - 1),
                            skip_group_check=True,
                        )
                kro = rtmp_pool.tile([128, SQW], F16, tag="kro", name="kro")
                rope(pq4, 0, kro)
                for dst_b in (0, 64):
                    nc.vector.tensor_copy(kt_aa[dst_b:dst_b + 64, 0:SQW], kro[0:64, :])
                    nc.vector.tensor_copy(kt_bb[dst_b:dst_b + 64, 0:SQW], kro[64:128, :])
                for m in range(NPAIR):
                    qrope[(m, 0)] = qrope_pool.tile(
                        [128, SQW], F16, tag="qr", name="qr"
                    )
                    rope(accs[m], 0, qrope[(m, 0)])
                qkv_chunk(0, 5, xt_t)

            # j=0 projection up front (chasing the DMAs), then
            # software-pipelined emission: attention(j) head-pairs are
            # interleaved with QKV(j+1) chunks and o_proj(j-1) row blocks so
            # the in-order PE queue always has independent work.
            qkv_j0()
            for j in range(NJ):
                xt_tiles.pop(j)
                if j + 1 < NJ:
                    load_xt(j + 1)
                for m in range(NPAIR):
                    attention_pair(j, m)
                    if j + 1 < NJ:
                        qkv_chunk(j + 1, m, xt_tiles[j + 1])
                        # K/V chunks early: the K-rope chain (scalar stage +
                        # DVE muls + kt copies) must land well before the
                        # next j's first QK reads kt
                        if m == 0:
                            qkv_chunk(j + 1, 4, xt_tiles[j + 1])
                        elif m == 1:
                            qkv_chunk(j + 1, 5, xt_tiles[j + 1])
                    if j > 0:
                        oproj_sc(j - 1, m)
                if j == NJ - 2:
                    # pull three of the last chunk's head-pairs forward so
                    # the tail window keeps the PE fed
                    attention_pair(NJ - 1, 0)
                    attention_pair(NJ - 1, 1)
                    attention_pair(NJ - 1, 2)
            for sc in range(4):
                oproj_sc(NJ - 1, sc)

    nc.finalize()
    _CACHED["nc"] = nc
    return nc


def _prep_inputs(cos, sin, hidden_states, per_head_scale, Wqkv, Wo):
    """Build the 8 per-core input maps (host-side, free)."""
    cos = np.asarray(cos, np.float32)
    sin = np.asarray(sin, np.float32)
    hs = np.asarray(hidden_states, np.float32)
    phs = np.asarray(per_head_scale, np.float32)
    Wqkv = np.asarray(Wqkv, np.float32)
    Wo = np.asarray(Wo, np.float32)

    cost = np.ascontiguousarray(np.vstack([cos.T, cos.T])).astype(np.float16)
    # sinp[base+32+i] = -sin[i] (multiplies q[base+32+i] -> t2[base+i]) and
    # sinp[base+i] = +sin[i] (multiplies q[base+i] -> t2[base+32+i]); both
    # DVE inputs then share a base partition.  Uses sin[0:32] == sin[32:64].
    s0 = sin.T[0:32]
    sints = np.ascontiguousarray(
        np.vstack([s0, -s0, s0, -s0])
    ).astype(np.float16)

    ident = np.eye(128, dtype=np.float32)
    tri = np.zeros((128, 128), np.float32)
    for p in range(128):
        tri[p, :p] = -BIG
    ebias_col = np.full((128, 1), EXP_BIAS, np.float32)
    ones_col = np.ones((128, 1), np.float32)
    consts = np.ascontiguousarray(
        np.concatenate([ident, tri, tri, ebias_col, ones_col], axis=1)
    ).astype(np.float16)

    xt_b = [np.ascontiguousarray(hs[b].T).astype(np.float16) for b in range(B)]

    in_maps = []
    for c in range(8):
        b, g = c // 4, c % 4
        hq0 = NH * g
        wq = Wqkv[hq0 * D:(hq0 + NH) * D, :].copy()
        for h in range(NH):
            wq[h * D:(h + 1) * D] *= (
                ROPE_MSCALE * SM_SCALE * phs[b, hq0 + h]
            )
        kv0 = H * D + NKV * g * D
        wk = Wqkv[kv0:kv0 + NKV * D, :] * ROPE_MSCALE
        v0 = (H + K) * D + NKV * g * D
        wv = Wqkv[v0:v0 + NKV * D, :]
        wqkv_c = np.ascontiguousarray(
            np.concatenate([wq, wk, wv], axis=0).T
        ).astype(np.float16)
        in_maps.append({
            "xt": xt_b[b],
            "wqkv": wqkv_c,
            "wo": np.ascontiguousarray(
                Wo[:, hq0 * D:(hq0 + NH) * D].T
            ).astype(np.float16),
            "cost": cost,
            "sints": sints,
            "consts": consts,
        })
    return in_maps


def kernel(cos, sin, hidden_states, per_head_scale, Wqkv, Wo, _trace=False):
    nc = _build()
    in_maps = _prep_inputs(cos, sin, hidden_states, per_head_scale, Wqkv, Wo)
    res = run_bass_kernel_spmd(nc, in_maps, core_ids=list(range(8)), trace=_trace)
    _CACHED["last_results"] = res
    out = np.stack([
        sum(res.results[b * 4 + g]["out"].astype(np.float32) for g in range(4))
        for b in range(B)
    ]).astype(np.float32)
    return out


# revision 43
# speedup vs baseline: 1.0053x; 1.0053x over previous
"""GQA attention (RoPE, causal, per-head q-scale) on 8 TRN2 NeuronCores.

Sharding: 2-way data-parallel over batch x 4-way tensor-parallel over heads.
Core c handles batch b=c//4 and head group g=c%4 (8 q heads, 2 kv heads).
Each core computes qkv-proj -> rope -> causal attention -> partial o_proj
(over its heads' columns of Wo); the host sums the 4 partials per batch.

All scalar factors (rope_mscale, sm_scale, per_head_scale) are folded into
the Wq/Wk rows on the host. Causal masking: fully-masked column blocks are
skipped (matmul widths trimmed to the causal extent); diagonal blocks get
-BIG added in PSUM via one identity x [tri|tri] matmul covering both heads
before the exp.

dtypes: the whole device datapath is fp16 (better mantissa than bf16 at the
same matmul/DVE speed); PSUM accumulation is f32.  exp uses a -5 bias so
probabilities stay inside fp16 range (cancels in the softmax ratio).
RoPE and normalization run on fp16 SBUF operands so the DVE gets its 2x
packed mode; psum->sbuf stagings ride the scalar engine.

Emission is software-pipelined: attention(j) head-pairs are interleaved
with QKV(j+1) chunks and o_proj(j-1) row blocks so the in-order PE queue
always has independent work while the scalar engine streams the exps.

Layouts on device (partition, free):
  xt      [hid, s]        hidden^T, streamed in 512-col chunks
  wqkv    [hid, 768]      [Wq(8 heads, scaled) | Wk(2 kv, scaled) | Wv].T
  q/k^T   [d*heads, s]    head-major rows; rope applied in this layout
  scores^T[sk, 2, sq]     per (pair, sk-chunk 128, sq-chunk 512) in PSUM
  exp^T   [sk, 2, sq]     SBUF fp16, fed as matmul rhs (both heads packed)
  Vaug    [sk, 128]       V rows (0:64) + 64 ones cols; PV matmul output
                          rows 64:128 then hold the softmax denominators
                          already broadcast over 64 partitions
  out^T   [2d, 2, sq]     PSUM accumulator per (pair, sq-chunk), both heads
  attn^T  [o(=2 heads), s] normalized fp16, lhsT for o_proj
  out     [s, hid_out]    partial o_proj result (fp16), one per core
"""

import sys, os

for _p in ("/opt/trn_rl_repo", "/root/.axon_site/_ro/trn_rl_repo"):
    if os.path.isdir(_p) and _p not in sys.path:
        sys.path.insert(0, _p)

import numpy as np

import concourse.bass as bass
import concourse.mybir as mybir
import concourse.tile as tile
from concourse import bacc
from concourse.bass_utils import run_bass_kernel_spmd

F32 = mybir.dt.float32
F16 = mybir.dt.float16
AF = mybir.ActivationFunctionType

B, S, HID = 2, 2048, 2048
H, K, D = 32, 8, 64
G = H // K
ROPE_MSCALE = 1.2
SM_SCALE = 1.0 / (D ** 0.5)
BIG = 30000.0
EXP_BIAS = -5.0  # exp(s-5) keeps fp16 probs finite; cancels in softmax

NH = 8           # q heads per core
NKV = 2          # kv heads per core
NPAIR = 4        # q head pairs per core
QO = NH * D      # 512 q rows
NK = HID // 128  # 16 contraction chunks
SQW = 512        # sq / xt chunk width
NJ = S // SQW    # 4 chunks
NSK = S // 128   # 16 sk chunks

_CACHED = {}


def _build():
    if "nc" in _CACHED:
        return _CACHED["nc"]

    nc = bacc.Bacc(None)

    xt_d = nc.declare_dram_parameter("xt", [HID, S], F16, isOutput=False)
    wqkv_d = nc.declare_dram_parameter("wqkv", [HID, 768], F16, isOutput=False)
    wo_d = nc.declare_dram_parameter("wo", [QO, HID], F16, isOutput=False)
    cost_d = nc.declare_dram_parameter("cost", [128, S], F16, isOutput=False)
    sints_d = nc.declare_dram_parameter("sints", [128, S], F16, isOutput=False)
    consts_d = nc.declare_dram_parameter("consts", [128, 386], F16, isOutput=False)
    out_d = nc.declare_dram_parameter("out", [S, HID], F16, isOutput=True)

    with tile.TileContext(nc) as tc:
        # ---------- long-lived pools ----------
        with (
            tc.tile_pool(name="consts", bufs=1) as consts_pool,
            tc.tile_pool(name="ktv", bufs=1) as ktv_pool,
            tc.tile_pool(name="qrope", bufs=10) as qrope_pool,
            tc.tile_pool(name="expt", bufs=6) as expt_pool,
            tc.tile_pool(name="attnt", bufs=8) as attnt_pool,
            tc.tile_pool(name="inv", bufs=2) as inv_pool,
            tc.tile_pool(name="wo", bufs=1) as wo_pool,
            tc.tile_pool(name="ost", bufs=4) as ost_pool,
            tc.tile_pool(name="wq", bufs=1) as wq_pool,
            tc.tile_pool(name="xt", bufs=16) as xt_pool,
            tc.tile_pool(name="cs", bufs=1) as cs_pool,
            tc.tile_pool(name="rtmp", bufs=2) as rtmp_pool,
            tc.tile_pool(name="psc", bufs=2, space="PSUM") as psc_pool,
            tc.tile_pool(name="pout2", bufs=1, space="PSUM") as pout2_pool,
            tc.tile_pool(name="pqkv", bufs=2, space="PSUM") as pqkv_pool,
        ):
            xt_r = xt_d.rearrange("(kc p) s -> p kc s", p=128)
            wq_r = wqkv_d.rearrange("(kc p) o -> p kc o", p=128)
            wo_r = wo_d.rearrange("(m p) h -> p m h", p=128)
            xt_tiles = {}

            def load_xt(j, interleave_with=None):
                t = []
                for kq in range(4):
                    tt = xt_pool.tile([128, 4, SQW], F16, tag="xt", name="xtt")
                    nc.sync.dma_start(
                        out=tt,
                        in_=xt_r[:, kq * 4:(kq + 1) * 4, j * SQW:(j + 1) * SQW],
                    )
                    t.append(tt)
                    if interleave_with is not None:
                        interleave_with(kq)
                xt_tiles[j] = t

            # prioritized loads: consts, then wqkv/xt(0) interleaved per 4-k
            # batch so the j=0 projection can chase the arriving DMAs; rope
            # tables mid-stream (needed ~20us in), Wo last.
            consts = consts_pool.tile([128, 386], F16, name="consts")
            nc.sync.dma_start(out=consts, in_=consts_d[:, :])

            # HAM warm-up: matmuls on the (tiny, first-to-arrive) consts tile
            # keep the PE busy during the big loads so the clock gate is at
            # 8/8 when real work arrives (~3.4us of activity needed).
            pwarm = pout2_pool.tile([128, 2, SQW], F32, tag="p2", name="pwarm")
            for _ in range(18):
                nc.tensor.matmul(
                    pwarm[:, 0, 0:384], consts[:, 0:128], consts[:, 0:384],
                    start=True, stop=True, skip_group_check=True,
                )
            costh = cs_pool.tile([128, S], F16, tag="cost", name="costh")
            sinth = cs_pool.tile([128, S], F16, tag="sints", name="sinth")
            wqt = wq_pool.tile([128, NK, 768], F16, name="wqt")
            xt0 = []
            for kq in range(4):
                nc.sync.dma_start(
                    out=wqt[:, kq * 4:(kq + 1) * 4, :],
                    in_=wq_r[:, kq * 4:(kq + 1) * 4, :],
                )
                tt = xt_pool.tile([128, 4, SQW], F16, tag="xt", name="xtt")
                nc.sync.dma_start(out=tt, in_=xt_r[:, kq * 4:(kq + 1) * 4, 0:SQW])
                xt0.append(tt)
                if kq == 1:
                    nc.sync.dma_start(out=costh, in_=cost_d[:, :])
                    nc.sync.dma_start(out=sinth, in_=sints_d[:, :])
            xt_tiles[0] = xt0
            wot = wo_pool.tile([128, NPAIR, HID], F16, name="wot")
            nc.sync.dma_start(out=wot, in_=wo_r)
            for jq in (1, 2, 3):
                load_xt(jq)

            identh = consts[:, 0:128]
            trih2 = consts[:, 128:384]  # two side-by-side -BIG lower triangles
            ebias = consts[:, 384:385]  # EXP_BIAS column for the activations

            kt_aa = ktv_pool.tile([128, S], F16, tag="ktaa", name="ktaa")
            kt_bb = ktv_pool.tile([128, S], F16, tag="ktbb", name="ktbb")
            # Vaug: cols 0:64 = V, cols 64:128 = 1.0 (sums -> rows 64:128 of PV out)
            vaug = [
                ktv_pool.tile([128, NSK, 128], F16, tag=f"vaug{i}", name=f"vaug{i}")
                for i in range(NKV)
            ]
            ones_col = consts[:, 385:386]
            for i in range(NKV):
                nc.vector.tensor_copy(
                    vaug[i][:, :, 64:128],
                    ones_col[:, None, :].broadcast_to([128, NSK, 64]),
                )

            qrope = {}   # (m, j) -> tile [128, SQW] f16
            attnt = {}   # (m, j) -> tile [128, SQW] f16
            _pending_pe = []  # deferred PE work (V transposes) to emit
                              # behind other matmuls so their input-staging
                              # latency never stalls the in-order PE queue

            def rope(psum_q, j, dst):
                """RoPE a [128, SQW] projected chunk (2 heads) into dst.
                q' = q*cos + swap_halves(q)*sin_signed.  The chunk is staged
                to fp16 SBUF by the scalar engine so every DVE op runs in the
                2x packed 16-bit mode and the QKV PSUM bank is released
                early.  sinth is partition-permuted + sign-folded on the host
                so both DVE inputs share a base partition (SBUF-SBUF rule);
                only the output is partition-shifted, which the DVE allows."""
                c0, c1 = j * SQW, (j + 1) * SQW
                qs = rtmp_pool.tile([128, SQW], F16, tag="qs", name="qs")
                nc.scalar.copy(qs, psum_q)
                t2 = rtmp_pool.tile([128, SQW], F16, tag="t2", name="t2")
                for base in (0, 64):
                    nc.vector.tensor_mul(
                        t2[base:base + 32, :], qs[base + 32:base + 64, :],
                        sinth[base + 32:base + 64, c0:c1],
                    )
                    nc.vector.tensor_mul(
                        t2[base + 32:base + 64, :], qs[base:base + 32, :],
                        sinth[base:base + 32, c0:c1],
                    )
                t4 = rtmp_pool.tile([128, SQW], F16, tag="t4", name="t4")
                nc.vector.tensor_mul(t4, qs, costh[:, c0:c1])
                nc.vector.tensor_add(dst, t2, t4)

            _done_pairs = set()

            def attention_pair(j, m):
                if (j, m) in _done_pairs:
                    return
                _done_pairs.add((j, m))
                nsk = 4 * (j + 1)
                kt = kt_aa if m < 2 else kt_bb
                va = vaug[m // 2]
                qr = qrope.pop((m, j))
                p2 = pout2_pool.tile([128, 2, SQW], F32, tag="p2", name="p2")
                pend = []  # staged (exp tile, sk)
                for sk in range(nsk):
                    # both heads' scores in one 2-bank tile; band blocks only
                    # compute the causally-valid columns
                    p1 = psc_pool.tile([128, 2, SQW], F32, tag="sc", name="sc")
                    band = sk >= 4 * j
                    off = (sk - 4 * j) * 128 if band else 0
                    for hi, hb in enumerate((0, 64)):
                        nc.tensor.matmul(
                            p1[:, hi, off:SQW],
                            kt[hb:hb + 64, sk * 128:(sk + 1) * 128],
                            qr[hb:hb + 64, off:SQW],
                            start=True,
                            stop=True,
                            skip_group_check=True,
                        )
                    if band:
                        # -BIG triangle onto both heads' diagonal blocks,
                        # added in PSUM by the vector engine (keeps the PE
                        # free; matmul accumulation is done by this point)
                        for hi in (0, 1):
                            nc.vector.tensor_add(
                                p1[:, hi, off:off + 128],
                                p1[:, hi, off:off + 128],
                                trih2[:, 0:128],
                            )
                    et = expt_pool.tile([128, 2, SQW], F16, tag="et", name="et")
                    nc.scalar.activation(
                        et[:, :, off:SQW], p1[:, :, off:SQW],
                        AF.Exp, bias=ebias,
                    )
                    pend.append((et, sk))
                    if sk == 1 and _pending_pe:
                        _pending_pe.pop(0)()
                    while len(pend) > 3 or (sk == nsk - 1 and pend):
                        et2, psk = pend.pop(0)
                        poff = (psk - 4 * j) * 128 if psk >= 4 * j else 0
                        for hi in (0, 1):
                            nc.tensor.matmul(
                                p2[:, hi, poff:SQW],
                                va[:, psk, :],
                                et2[:, hi, poff:SQW],
                                start=(psk == 0),
                                stop=(psk == nsk - 1),
                                skip_group_check=True,
                            )
                # normalize: attnT = out^T * (1/sums); sums come out of the
                # PV matmul pre-broadcast in psum rows 64:128
                # per-head chains: head B's sums copy rides the scalar engine
                # so both heads' recip+mul pipelines overlap on vector
                at = attnt_pool.tile([128, SQW], F16, tag="at", name="at")
                sums = inv_pool.tile([64, 2, SQW], F32, tag="sums", name="sums")
                invb = inv_pool.tile([64, 2, SQW], F32, tag="invb", name="invb")
                nc.vector.tensor_copy(sums[:, 0, :], p2[64:128, 0, :])
                nc.scalar.copy(sums[:, 1, :], p2[64:128, 1, :])
                nc.vector.reciprocal_approx_fast(out=invb[:, 0, :], in_=sums[:, 0, :])
                nc.vector.tensor_mul(at[0:64, :], p2[0:64, 0, :], invb[:, 0, :])
                nc.vector.reciprocal_approx_fast(out=invb[:, 1, :], in_=sums[:, 1, :])
                nc.vector.tensor_mul(at[64:128, :], p2[0:64, 1, :], invb[:, 1, :])
                attnt[(m, j)] = at

            def oproj_sc(j, sc):
                orow = None
                for hc in range(HID // 512):
                    po = pqkv_pool.tile([128, 512], F32, tag="qkv", name="po")
                    for m in range(NPAIR):
                        nc.tensor.matmul(
                            po,
                            attnt[(m, j)][:, sc * 128:(sc + 1) * 128],
                            wot[:, m, hc * 512:(hc + 1) * 512],
                            start=(m == 0),
                            stop=(m == NPAIR - 1),
                        )
                    if orow is None:
                        # allocated just before the first evacuation so the
                        # pool-slot wait binds to the vector copy, not the
                        # o_proj matmuls
                        orow = ost_pool.tile([128, HID], F16, tag="orow", name="orow")
                    if hc % 2 == 0:
                        nc.vector.tensor_copy(orow[:, hc * 512:(hc + 1) * 512], po)
                    else:
                        nc.scalar.copy(orow[:, hc * 512:(hc + 1) * 512], po)
                r0 = j * SQW + sc * 128
                # alternate trigger queues so the tail's bunched row DMAs
                # don't serialize on one engine's DMA queue
                eng = nc.gpsimd if sc % 2 == 0 else nc.sync
                eng.dma_start(out=out_d[r0:r0 + 128, :], in_=orow)
                if sc == SQW // 128 - 1:
                    for m in range(NPAIR):
                        attnt.pop((m, j))

            def qkv_chunk(j, m, xt_t):
                if m == 5:
                    # V projected in the [d, s] orientation (512-col moving,
                    # weight loads hidden), then PE-transposed per 128-block
                    # into the [sk, d] layout Vaug needs.
                    pv = pqkv_pool.tile([128, SQW], F32, tag="qkv", name="pv")
                    for k in range(NK):
                        nc.tensor.matmul(
                            pv,
                            wqt[:, k, 640:768],
                            xt_t[k // 4][:, k % 4, :],
                            start=(k == 0),
                            stop=(k == NK - 1),
                        )
                    vt = rtmp_pool.tile([128, SQW], F16, tag="vt", name="vt")
                    nc.scalar.copy(vt, pv)

                    def _finish_v(j=j, vt=vt):
                        tp = psc_pool.tile([128, 4, 128], F16, tag="sc", name="tp")
                        for blk in range(4):
                            nc.tensor.transpose(
                                tp[:, blk, :], vt[:, blk * 128:(blk + 1) * 128],
                                identh,
                            )
                        sk0 = (j * SQW) // 128
                        for i in range(NKV):
                            for blk in range(4):
                                nc.vector.tensor_copy(
                                    vaug[i][:, sk0 + blk, 0:64],
                                    tp[:, blk, i * 64:(i + 1) * 64],
                                )
                    _pending_pe.append(_finish_v)
                    return
                pq = pqkv_pool.tile([128, SQW], F32, tag="qkv", name="pqkv")
                for k in range(NK):
                    nc.tensor.matmul(
                        pq,
                        wqt[:, k, m * 128:(m + 1) * 128],
                        xt_t[k // 4][:, k % 4, :],
                        start=(k == 0),
                        stop=(k == NK - 1),
                    )
                if m < NPAIR:
                    qrope[(m, j)] = qrope_pool.tile(
                        [128, SQW], F16, tag="qr", name="qr"
                    )
                    rope(pq, j, qrope[(m, j)])
                else:  # m == 4: K
                    kro = rtmp_pool.tile([128, SQW], F16, tag="kro", name="kro")
                    rope(pq, j, kro)
                    c0, c1 = j * SQW, (j + 1) * SQW
                    for dst_b in (0, 64):
                        nc.vector.tensor_copy(
                            kt_aa[dst_b:dst_b + 64, c0:c1], kro[0:64, :]
                        )
                        nc.vector.tensor_copy(
                            kt_bb[dst_b:dst_b + 64, c0:c1], kro[64:128, :]
                        )

            def qkv_j0():
                """j=0 projection with k OUTER across all five chunks so the
                matmuls chase the arriving wqt/xt0 DMA batches.  The five
                f32 accumulators land in five distinct psum banks (the q
                pairs borrow the two psc slots' bank halves), so the
                interleaved starts never clear each other's has_written."""
                pq4 = pqkv_pool.tile([128, SQW], F32, tag="qkv", name="pqkv")
                pA = psc_pool.tile([128, 2, SQW], F32, tag="sc", name="scA")
                pB = psc_pool.tile([128, 2, SQW], F32, tag="sc", name="scB")
                accs = [pA[:, 0, :], pA[:, 1, :], pB[:, 0, :], pB[:, 1, :]]
                xt_t = xt_tiles[0]
                for k in range(NK):
                    nc.tensor.matmul(
                        pq4,
                        wqt[:, k, 512:640],
                        xt_t[k // 4][:, k % 4, :],
                        start=(k == 0),
                        stop=(k == NK - 1),
                        skip_group_check=True,
                    )
                    for m in range(NPAIR):
                        nc.tensor.matmul(
                            accs[m],
                            wqt[:, k, m * 128:(m + 1) * 128],
                            xt_t[k // 4][:, k % 4, :],
                            start=(k == 0),
                            stop=(k == NK - 1),
                            skip_group_check=True,
                        )
                kro = rtmp_pool.tile([128, SQW], F16, tag="kro", name="kro")
                rope(pq4, 0, kro)
                for dst_b in (0, 64):
                    nc.vector.tensor_copy(kt_aa[dst_b:dst_b + 64, 0:SQW], kro[0:64, :])
                    nc.vector.tensor_copy(kt_bb[dst_b:dst_b + 64, 0:SQW], kro[64:128, :])
                for m in range(NPAIR):
                    qrope[(m, 0)] = qrope_pool.tile(
                        [128, SQW], F16, tag="qr", name="qr"
                    )
                    rope(accs[m], 0, qrope[(m, 0)])
                qkv_chunk(0, 5, xt_t)

            # j=0 projection up front (chasing the DMAs), then ALL K/V
            # chunks, then attention pairs in DESCENDING j order: the
            # heaviest exp chains (j=3, 16 sk-chunks each) run early where
            # plenty of QKV/o_proj work hides the scalar latency, and the
            # tail pair (j=0) has only 4 exps, so the closing
            # norm -> o_proj(0) chain is short and never lets HAM dip.
            qkv_j0()
            for jq in (1, 2, 3):
                qkv_chunk(jq, 4, xt_tiles[jq])
                qkv_chunk(jq, 5, xt_tiles[jq])
            qkv_chunk(3, 0, xt_tiles[3])
            qkv_chunk(3, 1, xt_tiles[3])
            while _pending_pe:
                _pending_pe.pop(0)()
            qprod = [(3, 2), (3, 3)] + [(2, m) for m in range(4)] \
                + [(1, m) for m in range(4)]
            for jj in (3, 2, 1, 0):
                for m in range(NPAIR):
                    attention_pair(jj, m)
                    if qprod:
                        jq, mq = qprod.pop(0)
                        qkv_chunk(jq, mq, xt_tiles[jq])
                    if jj < 3:
                        oproj_sc(jj + 1, m)
            for sc in range(4):
                oproj_sc(0, sc)

    nc.finalize()
    _CACHED["nc"] = nc
    return nc


def _prep_inputs(cos, sin, hidden_states, per_head_scale, Wqkv, Wo):
    """Build the 8 per-core input maps (host-side, free)."""
    cos = np.asarray(cos, np.float32)
    sin = np.asarray(sin, np.float32)
    hs = np.asarray(hidden_states, np.float32)
    phs = np.asarray(per_head_scale, np.float32)
    Wqkv = np.asarray(Wqkv, np.float32)
    Wo = np.asarray(Wo, np.float32)

    cost = np.ascontiguousarray(np.vstack([cos.T, cos.T])).astype(np.float16)
    # sinp[base+32+i] = -sin[i] (multiplies q[base+32+i] -> t2[base+i]) and
    # sinp[base+i] = +sin[i] (multiplies q[base+i] -> t2[base+32+i]); both
    # DVE inputs then share a base partition.  Uses sin[0:32] == sin[32:64].
    s0 = sin.T[0:32]
    sints = np.ascontiguousarray(
        np.vstack([s0, -s0, s0, -s0])
    ).astype(np.float16)

    ident = np.eye(128, dtype=np.float32)
    tri = np.zeros((128, 128), np.float32)
    for p in range(128):
        tri[p, :p] = -BIG
    ebias_col = np.full((128, 1), EXP_BIAS, np.float32)
    ones_col = np.ones((128, 1), np.float32)
    consts = np.ascontiguousarray(
        np.concatenate([ident, tri, tri, ebias_col, ones_col], axis=1)
    ).astype(np.float16)

    xt_b = [np.ascontiguousarray(hs[b].T).astype(np.float16) for b in range(B)]

    in_maps = []
    for c in range(8):
        b, g = c // 4, c % 4
        hq0 = NH * g
        wq = Wqkv[hq0 * D:(hq0 + NH) * D, :].copy()
        for h in range(NH):
            wq[h * D:(h + 1) * D] *= (
                ROPE_MSCALE * SM_SCALE * phs[b, hq0 + h]
            )
        kv0 = H * D + NKV * g * D
        wk = Wqkv[kv0:kv0 + NKV * D, :] * ROPE_MSCALE
        v0 = (H + K) * D + NKV * g * D
        wv = Wqkv[v0:v0 + NKV * D, :]
        wqkv_c = np.ascontiguousarray(
            np.concatenate([wq, wk, wv], axis=0).T
        ).astype(np.float16)
        in_maps.append({
            "xt": xt_b[b],
            "wqkv": wqkv_c,
            "wo": np.ascontiguousarray(
                Wo[:, hq0 * D:(hq0 + NH) * D].T
            ).astype(np.float16),
            "cost": cost,
            "sints": sints,
            "consts": consts,
        })
    return in_maps


def kernel(cos, sin, hidden_states, per_head_scale, Wqkv, Wo, _trace=False):
    nc = _build()
    in_maps = _prep_inputs(cos, sin, hidden_states, per_head_scale, Wqkv, Wo)
    res = run_bass_kernel_spmd(nc, in_maps, core_ids=list(range(8)), trace=_trace)
    _CACHED["last_results"] = res
    out = np.stack([
        sum(res.results[b * 4 + g]["out"].astype(np.float32) for g in range(4))
        for b in range(B)
    ]).astype(np.float32)
    return out
